# revision 4
# baseline (speedup 1.0000x reference)
"""GAT layer (nn_GAT_28784870818245) on 8 Trainium2 NeuronCores.

Data-parallel over the batch dim: core b computes batch element b.

Per-core dataflow (N=2048 rows, F=256 features, P=128 partitions, T=16 row
tiles):

  Wh        = h @ W                       PE, via PE-transposed h tiles
  wh1/wh2   = Wh @ a1 / Wh @ a2           folded into the same matmul: the
                                          host passes Wext = [W | W@a1 | W@a2]
  z[i,j]    = wh1[i] + wh2[j]
  p         = exp(leaky_relu(z, 0.2))     computed as max(exp(z), exp(0.2 z))
                                          (exp is monotone, so this is exact);
                                          exp's free affine (scale*in + bias)
                                          supplies both branches from a single
                                          broadcast wh2 row: ACT only
  p_m       = p * adj                     GPSIMD (frees DVE; adj arrives as f32)
  softmax   denominators come for free: each h' matmul block carries a ones
            column, so rowsum accumulates in PSUM next to h'
  att       = p_m * (1/rowsum)            DVE tensor_scalar (2x mode)
  h'        = attT.T @ [Wh | 1]           PE, attT via PE transposes
  out       = h + elu(h' / rowsum)        elu(x) = min(exp(x),1) - 1 + relu(x)

No max-subtraction in the softmax: z is bounded (~|z| < 25 for this input
distribution, exp(25) ~ 7e10 fits comfortably in f32), and skipping it is
mathematically identical.

Softmax row maxima over masked-out entries are unneeded because masking
happens on the exp'd values (exp(-inf) == 0 semantics of the reference).
"""

import numpy as np
from contextlib import ExitStack

import concourse.bass as bass
import concourse.mybir as mybir
import concourse.tile as tile
from concourse.bass_utils import run_bass_kernel_spmd

F32 = mybir.dt.float32
B, N, F = 8, 2048, 256
P = 128
T = N // P            # 16 row tiles per core
WE = F + 2            # Wext columns: W | W@a1 | W@a2
FB = F + 1            # h' matmul rhs block: Wh | ones

_MAXW_PER_INST = 1


def _split_sync_waits(nc, limit=_MAXW_PER_INST):
    """walrus in this container rejects instructions with more than one
    sync-wait command; move excess waits onto same-engine NoOps inserted
    right before the instruction (waits run strictly earlier on the same
    engine, so ordering is preserved)."""
    ctr = 0
    for blk in nc.m.functions[0].blocks:
        new = []
        for inst in blk.instructions:
            si = inst.sync_info
            if si is not None and si.on_wait and len(si.on_wait) > limit:
                waits = list(si.on_wait)
                excess, keep = waits[:-limit], waits[-limit:]
                for k in range(0, len(excess), limit):
                    nop = mybir.InstNoOp(
                        name=f"I-waitsplit-{ctr}", ins=[], outs=[]
                    )
                    ctr += 1
                    nop.engine = inst.engine
                    nop.sync_info = mybir.SyncInfo(
                        on_wait=list(excess[k : k + limit]), on_update=[]
                    )
                    nop.bass_nofuse = True
                    new.append(nop)
                del si.on_wait[:]
                si.on_wait.extend(keep)
            new.append(inst)
        blk.instructions[:] = new


def _build():
    AF = mybir.ActivationFunctionType
    OP = mybir.AluOpType

    nc = bass.Bass("TRN2", target_bir_lowering=False, debug=False, num_devices=B)

    def copy_alt(k, out, in_):
        # alternate psum->sbuf copies between DVE and ACT to balance load
        if k % 2:
            nc.vector.tensor_copy(out, in_)
        else:
            nc.scalar.copy(out, in_)
    h_d = nc.dram_tensor("h", [N, F], F32, kind="ExternalInput").ap()
    adj_d = nc.dram_tensor("adjf", [N, N], F32, kind="ExternalInput").ap()
    wext_d = nc.dram_tensor("wext", [F, WE], F32, kind="ExternalInput").ap()
    ident_d = nc.dram_tensor("ident", [P, P], F32, kind="ExternalInput").ap()
    out_d = nc.dram_tensor("out", [N, F], F32, kind="ExternalOutput").ap()
    att_d = nc.dram_tensor("att", [N, N], F32, kind="ExternalOutput").ap()
    scr_d = nc.dram_tensor("scr", [N], F32)

    with tile.TileContext(nc) as tc, ExitStack() as ctx:
        const = ctx.enter_context(tc.tile_pool(name="const", bufs=1))
        wext_sb = const.tile([P, 2 * WE], F32)
        ident_sb = const.tile([P, P], F32)
        ones1 = const.tile([1, P], F32)
        h_sb = const.tile([P, T * F], F32)
        whb = const.tile([P, T * FB], F32)     # [Wh_j | 1] blocks, rhs of h' mm
        wh2b = const.tile([P, N], F32)         # wh2 row broadcast to all parts
        wh1c = const.tile([P, T], F32)
        wh1c02 = const.tile([P, T], F32)
        wh2c = const.tile([P, T], F32)

        nc.vector.memset(ones1, 1.0)
        for c in range(2):
            nc.sync.dma_start(
                out=wext_sb[:, c * WE : (c + 1) * WE],
                in_=wext_d[c * P : (c + 1) * P, :],
            )
        nc.sync.dma_start(out=ident_sb, in_=ident_d)
        for i in range(T):
            nc.sync.dma_start(
                out=h_sb[:, i * F : (i + 1) * F], in_=h_d[i * P : (i + 1) * P, :]
            )

        # ---- setup: hT, Wh = h @ Wext, wh2 broadcast ----
        with tc.tile_pool(name="hT", bufs=1) as hTp, \
             tc.tile_pool(name="setps", bufs=2, space="PSUM") as setps, \
             tc.tile_pool(name="tmp", bufs=2) as tmpp:
            hT = [
                hTp.tile([P, N], F32, tag=f"hT{c}", name=f"hT{c}")
                for c in range(2)
            ]
            for i in range(T):
                for c in range(2):
                    tp = setps.tile([P, P], F32, tag="htps")
                    nc.tensor.transpose(
                        tp, h_sb[:, i * F + c * P : i * F + (c + 1) * P], ident_sb
                    )
                    copy_alt(2 * i + c + 1, hT[c][:, i * P : (i + 1) * P], tp)

            for i in range(T):
                pw = setps.tile([P, WE], F32, tag="whps")
                for c in range(2):
                    nc.tensor.matmul(
                        pw,
                        lhsT=hT[c][:, i * P : (i + 1) * P],
                        rhs=wext_sb[:, c * WE : (c + 1) * WE],
                        start=(c == 0),
                        stop=(c == 1),
                    )
                copy_alt(i + 1, whb[:, i * FB : i * FB + F], pw[:, 0:F])
                nc.vector.tensor_copy(wh1c[:, i : i + 1], pw[:, F : F + 1])
                nc.vector.tensor_copy(wh2c[:, i : i + 1], pw[:, F + 1 : F + 2])
                nc.vector.memset(whb[:, i * FB + F : (i + 1) * FB], 1.0)

            nc.vector.tensor_scalar_mul(wh1c02, wh1c, 0.2)

            # wh2 column-per-tile -> one [1, N] row (via PE transpose and a
            # DRAM bounce) -> broadcast to [P, N] with a K=1 ones matmul
            tps = setps.tile([T, P], F32, tag="w2t")
            nc.tensor.transpose(tps, wh2c, ident_sb)
            stage = tmpp.tile([T, P], F32)
            nc.vector.tensor_copy(stage, tps)
            nc.sync.dma_start(out=scr_d.ap(), in_=stage)
            row1 = tmpp.tile([1, N], F32)
            nc.sync.dma_start(
                out=row1, in_=scr_d.ap().rearrange("(p n) -> p n", p=1)
            )
            for q in range(4):
                bc = setps.tile([P, 512], F32, tag="bcps")
                nc.tensor.matmul(
                    bc,
                    lhsT=ones1,
                    rhs=row1[0:1, q * 512 : (q + 1) * 512],
                    start=True,
                    stop=True,
                )
                copy_alt(q + 1, wh2b[:, q * 512 : (q + 1) * 512], bc)

        # ---- main loop over row tiles ----
        adjp = ctx.enter_context(tc.tile_pool(name="adj", bufs=2))
        q1p = ctx.enter_context(tc.tile_pool(name="q1", bufs=2))
        q2p = ctx.enter_context(tc.tile_pool(name="q2", bufs=2))
        pmp = ctx.enter_context(tc.tile_pool(name="pm", bufs=2))
        pm2p = ctx.enter_context(tc.tile_pool(name="pm2", bufs=2))
        pmTp = ctx.enter_context(tc.tile_pool(name="pmT", bufs=2))
        attp = ctx.enter_context(tc.tile_pool(name="attn", bufs=2))
        smallp = ctx.enter_context(tc.tile_pool(name="small", bufs=4))
        elup = ctx.enter_context(tc.tile_pool(name="elu", bufs=3))
        trps = ctx.enter_context(tc.tile_pool(name="trps", bufs=2, space="PSUM"))
        hpps = ctx.enter_context(tc.tile_pool(name="hpps", bufs=2, space="PSUM"))

        for i in range(T):
            adj_t = adjp.tile([P, N], F32)
            nc.sync.dma_start(out=adj_t, in_=adj_d[i * P : (i + 1) * P, :])

            # p = exp(leaky_relu(wh1[i] + wh2[j])) = max(exp(z), exp(0.2 z))
            q1 = q1p.tile([P, N], F32)
            nc.scalar.activation(
                q1, wh2b, AF.Exp, bias=wh1c[:, i : i + 1], scale=1.0
            )
            q2 = q2p.tile([P, N], F32)
            nc.scalar.activation(
                q2, wh2b, AF.Exp, bias=wh1c02[:, i : i + 1], scale=0.2
            )
            pm = pmp.tile([P, N], F32)
            nc.vector.tensor_tensor(out=pm, in0=q1, in1=q2, op=OP.max)
            pm2 = pm2p.tile([P, N], F32)
            nc.gpsimd.tensor_tensor(out=pm2, in0=pm, in1=adj_t, op=OP.mult)

            # transpose p_m into [j, i] blocks for the h' contraction
            pmT = pmTp.tile([P, N], F32)
            for g in range(4):
                tp = trps.tile([P, 512], F32, tag="tr")
                for s in range(4):
                    j = 4 * g + s
                    nc.tensor.transpose(
                        tp[:, s * P : (s + 1) * P],
                        pm2[:, j * P : (j + 1) * P],
                        ident_sb,
                    )
                copy_alt(g + 1, pmT[:, g * 512 : (g + 1) * 512], tp)

            # h'[i] (cols 0:F) and rowsum (col F) accumulate together
            hp = hpps.tile([P, FB], F32)
            for j in range(T):
                nc.tensor.matmul(
                    hp,
                    lhsT=pmT[:, j * P : (j + 1) * P],
                    rhs=whb[:, j * FB : (j + 1) * FB],
                    start=(j == 0),
                    stop=(j == T - 1),
                )
            rec = smallp.tile([P, 1], F32)
            nc.vector.reciprocal(rec, hp[:, F : F + 1])

            att_t = attp.tile([P, N], F32)
            nc.vector.tensor_scalar_mul(att_t, pm2, rec[:, 0:1])
            nc.sync.dma_start(out=att_d[i * P : (i + 1) * P, :], in_=att_t)

            # out = h + elu(h' * rec); elu(x) = min(exp(x),1) - 1 + relu(x)
            xe = elup.tile([P, F], F32, tag="xe")
            nc.scalar.activation(
                xe, hp[:, 0:F], AF.Exp, bias=0.0, scale=rec[:, 0:1]
            )
            xp = elup.tile([P, F], F32, tag="xp")
            nc.vector.tensor_scalar(
                out=xp, in0=hp[:, 0:F], scalar1=rec[:, 0:1], scalar2=0.0,
                op0=OP.mult, op1=OP.max,
            )
            tm = elup.tile([P, F], F32, tag="tm")
            nc.vector.tensor_scalar(
                out=tm, in0=xe, scalar1=1.0, scalar2=-1.0,
                op0=OP.min, op1=OP.add,
            )
            s1 = elup.tile([P, F], F32, tag="s1")
            nc.vector.tensor_tensor(out=s1, in0=tm, in1=xp, op=OP.add)
            ou = elup.tile([P, F], F32, tag="ou")
            nc.gpsimd.tensor_tensor(
                out=ou, in0=s1, in1=h_sb[:, i * F : (i + 1) * F], op=OP.add
            )
            nc.sync.dma_start(out=out_d[i * P : (i + 1) * P, :], in_=ou)

    _split_sync_waits(nc)
    return nc


_NC = None


def kernel(h, adj, W, a):
    global _NC
    if _NC is None:
        _NC = _build()
    h = np.ascontiguousarray(np.asarray(h, dtype=np.float32))
    adj = np.asarray(adj)
    W = np.asarray(W, dtype=np.float32)
    a = np.asarray(a, dtype=np.float32)
    wext = np.concatenate([W, W @ a[:F], W @ a[F:]], axis=1).astype(np.float32)
    ident = np.eye(P, dtype=np.float32)
    in_maps = [
        {
            "h": h[b],
            "adjf": np.ascontiguousarray(adj[b].astype(np.float32)),
            "wext": wext,
            "ident": ident,
        }
        for b in range(B)
    ]
    res = run_bass_kernel_spmd(_NC, in_maps, list(range(B)))
    out = np.stack([res.results[b]["out"] for b in range(B)])
    att = np.stack([res.results[b]["att"] for b in range(B)])
    return out, att


# revision 7
# speedup vs baseline: 1.1514x; 1.1514x over previous
"""GAT layer (nn_GAT_28784870818245) on 8 Trainium2 NeuronCores.

Data-parallel over the batch dim: core b computes batch element b.

Per-core dataflow (N=2048 rows, F=256 features, P=128 partitions, T=16 row
tiles):

  Wh         = h @ [W | W@a1 | W@a2]      PE; host passes Wext (weight folding)
                                          and hT = h.T (no on-chip h transpose)
  z[i,j]     = wh1[i] + wh2[j]            wh2 broadcast to a [P, N] tile once,
                                          wh1 rides the activation bias
  e          = leaky_relu(z, 0.2)         ACT Prelu (alpha carries the slope)
  p          = exp(e)                     ACT (no max-subtraction: |z| < ~25,
                                          exp fits f32; identical math)
  p_m        = p * adj                    GPSIMD; adj ships as uint8 (4x less DMA)
  rowsum     rides the h' matmul: each [Wh_j | 1] rhs block carries a ones col
  att        = p_m * (1/rowsum)           DVE tensor_scalar (2x mode)
  h'         = pmT.T @ [Wh | 1]           PE; pmT via PE transposes
  out        = h + elu(h'/rowsum)         elu(x) = min(exp(x),1) - 1 + relu(x)

DMA: loads issue from SP (sync), the att store from ACT and the out store
from GPSIMD, so store waits never block load issue order on one sequencer.
"""

import numpy as np
from contextlib import ExitStack

import concourse.bass as bass
import concourse.mybir as mybir
import concourse.tile as tile
from concourse.bass_utils import run_bass_kernel_spmd

F32 = mybir.dt.float32
U8 = mybir.dt.uint8
B, N, F = 8, 2048, 256
P = 128
T = N // P            # 16 row tiles per core
WE = F + 2            # Wext columns: W | W@a1 | W@a2
FB = F + 1            # h' matmul rhs block: Wh | ones

_MAXW_PER_INST = 1


def _split_sync_waits(nc, limit=_MAXW_PER_INST):
    """walrus in this container rejects instructions with more than one
    sync-wait command; move excess waits onto same-engine NoOps inserted
    right before the instruction (waits run strictly earlier on the same
    engine, so ordering is preserved)."""
    ctr = 0
    for blk in nc.m.functions[0].blocks:
        new = []
        for inst in blk.instructions:
            si = inst.sync_info
            if si is not None and si.on_wait and len(si.on_wait) > limit:
                waits = list(si.on_wait)
                excess, keep = waits[:-limit], waits[-limit:]
                for k in range(0, len(excess), limit):
                    nop = mybir.InstNoOp(
                        name=f"I-waitsplit-{ctr}", ins=[], outs=[]
                    )
                    ctr += 1
                    nop.engine = inst.engine
                    nop.sync_info = mybir.SyncInfo(
                        on_wait=list(excess[k : k + limit]), on_update=[]
                    )
                    nop.bass_nofuse = True
                    new.append(nop)
                del si.on_wait[:]
                si.on_wait.extend(keep)
            new.append(inst)
        blk.instructions[:] = new


def _build(repeat=1):
    AF = mybir.ActivationFunctionType
    OP = mybir.AluOpType

    nc = bass.Bass("TRN2", target_bir_lowering=False, debug=False, num_devices=B)

    def copy_alt(k, out, in_):
        # psum->sbuf copies; k selects the engine to balance load
        if k % 4 != 0:
            nc.vector.tensor_copy(out, in_)
        else:
            nc.scalar.copy(out, in_)

    h_d = nc.dram_tensor("h", [N, F], F32, kind="ExternalInput").ap()
    hT_d = nc.dram_tensor("hT", [F, N], F32, kind="ExternalInput").ap()
    adj_d = nc.dram_tensor("adj8", [N, N], U8, kind="ExternalInput").ap()
    wext_d = nc.dram_tensor("wext", [F, WE], F32, kind="ExternalInput").ap()
    ident_d = nc.dram_tensor("ident", [P, P], F32, kind="ExternalInput").ap()
    out_d = nc.dram_tensor("out", [N, F], F32, kind="ExternalOutput").ap()
    att_d = nc.dram_tensor("att", [N, N], F32, kind="ExternalOutput").ap()
    scr_d = nc.dram_tensor("scr", [N], F32)

    with tile.TileContext(nc) as tc, ExitStack() as ctx:
        const = ctx.enter_context(tc.tile_pool(name="const", bufs=1))
        wext_sb = const.tile([P, 2 * WE], F32)
        ident_sb = const.tile([P, P], F32)
        ones1 = const.tile([1, P], F32)
        h_sb = const.tile([P, T * F], F32)
        whb = const.tile([P, T * FB], F32)     # [Wh_j | 1] blocks, rhs of h' mm
        wh2b = const.tile([P, N], F32)         # wh2 row broadcast to all parts
        wh1c = const.tile([P, T], F32)
        wh2c = const.tile([P, T], F32)

        nc.vector.memset(ones1, 1.0)
        for c in range(2):
            nc.sync.dma_start(
                out=wext_sb[:, c * WE : (c + 1) * WE],
                in_=wext_d[c * P : (c + 1) * P, :],
            )
        nc.sync.dma_start(out=ident_sb, in_=ident_d)
        for i in range(T):
            nc.sync.dma_start(
                out=h_sb[:, i * F : (i + 1) * F], in_=h_d[i * P : (i + 1) * P, :]
            )

        # ---- setup: Wh = h @ Wext (hT from host), wh2 broadcast ----
        with tc.tile_pool(name="hT", bufs=1) as hTp, \
             tc.tile_pool(name="setps", bufs=2, space="PSUM") as setps, \
             tc.tile_pool(name="tmp", bufs=2) as tmpp:
            hT = [
                hTp.tile([P, N], F32, tag=f"hT{c}", name=f"hT{c}")
                for c in range(2)
            ]
            for c in range(2):
                nc.sync.dma_start(out=hT[c], in_=hT_d[c * P : (c + 1) * P, :])

            for i in range(T):
                pw = setps.tile([P, WE], F32, tag="whps")
                for c in range(2):
                    nc.tensor.matmul(
                        pw,
                        lhsT=hT[c][:, i * P : (i + 1) * P],
                        rhs=wext_sb[:, c * WE : (c + 1) * WE],
                        start=(c == 0),
                        stop=(c == 1),
                    )
                copy_alt(i + 1, whb[:, i * FB : i * FB + F], pw[:, 0:F])
                nc.vector.tensor_copy(wh1c[:, i : i + 1], pw[:, F : F + 1])
                nc.vector.tensor_copy(wh2c[:, i : i + 1], pw[:, F + 1 : F + 2])
                nc.vector.memset(whb[:, i * FB + F : (i + 1) * FB], 1.0)

            # wh2 column-per-tile -> one [1, N] row (via PE transpose and a
            # DRAM bounce) -> broadcast to [P, N] with a K=1 ones matmul
            tps = setps.tile([T, P], F32, tag="w2t")
            nc.tensor.transpose(tps, wh2c, ident_sb)
            stage = tmpp.tile([T, P], F32)
            nc.vector.tensor_copy(stage, tps)
            nc.sync.dma_start(out=scr_d.ap(), in_=stage)
            row1 = tmpp.tile([1, N], F32)
            nc.sync.dma_start(
                out=row1, in_=scr_d.ap().rearrange("(p n) -> p n", p=1)
            )
            for q in range(4):
                bc = setps.tile([P, 512], F32, tag="bcps")
                nc.tensor.matmul(
                    bc,
                    lhsT=ones1,
                    rhs=row1[0:1, q * 512 : (q + 1) * 512],
                    start=True,
                    stop=True,
                )
                copy_alt(q + 1, wh2b[:, q * 512 : (q + 1) * 512], bc)

        # ---- main loop over row tiles ----
        adjp = ctx.enter_context(tc.tile_pool(name="adj", bufs=4))
        ep = ctx.enter_context(tc.tile_pool(name="escore", bufs=2))
        pp = ctx.enter_context(tc.tile_pool(name="pexp", bufs=2))
        pm2p = ctx.enter_context(tc.tile_pool(name="pm2", bufs=2))
        pmTp = ctx.enter_context(tc.tile_pool(name="pmT", bufs=2))
        attp = ctx.enter_context(tc.tile_pool(name="attn", bufs=2))
        smallp = ctx.enter_context(tc.tile_pool(name="small", bufs=4))
        elup = ctx.enter_context(tc.tile_pool(name="elu", bufs=3))
        trps = ctx.enter_context(tc.tile_pool(name="trps", bufs=2, space="PSUM"))
        hpps = ctx.enter_context(tc.tile_pool(name="hpps", bufs=2, space="PSUM"))

        for rep in range(repeat):
          for i in range(T):
            adj_t = adjp.tile([P, N], U8)
            nc.sync.dma_start(out=adj_t, in_=adj_d[i * P : (i + 1) * P, :])

            # p = exp(leaky_relu(wh1[i] + wh2[j], 0.2))
            e_t = ep.tile([P, N], F32)
            nc.scalar.activation(
                e_t, wh2b, AF.Prelu, bias=wh1c[:, i : i + 1], scale=1.0,
                alpha=0.2,
            )
            p_t = pp.tile([P, N], F32)
            nc.scalar.activation(p_t, e_t, AF.Exp, bias=0.0, scale=1.0)
            pm2 = pm2p.tile([P, N], F32)
            nc.gpsimd.tensor_tensor(out=pm2, in0=p_t, in1=adj_t, op=OP.mult)

            # transpose p_m into [j, i] blocks for the h' contraction
            pmT = pmTp.tile([P, N], F32)
            for g in range(4):
                tp = trps.tile([P, 512], F32, tag="tr")
                for s in range(4):
                    j = 4 * g + s
                    nc.tensor.transpose(
                        tp[:, s * P : (s + 1) * P],
                        pm2[:, j * P : (j + 1) * P],
                        ident_sb,
                    )
                copy_alt(g, pmT[:, g * 512 : (g + 1) * 512], tp)

            # h'[i] (cols 0:F) and rowsum (col F) accumulate together
            hp = hpps.tile([P, FB], F32)
            for j in range(T):
                nc.tensor.matmul(
                    hp,
                    lhsT=pmT[:, j * P : (j + 1) * P],
                    rhs=whb[:, j * FB : (j + 1) * FB],
                    start=(j == 0),
                    stop=(j == T - 1),
                )
            rec = smallp.tile([P, 1], F32)
            nc.vector.reciprocal(rec, hp[:, F : F + 1])

            att_t = attp.tile([P, N], F32)
            nc.vector.tensor_scalar_mul(att_t, pm2, rec[:, 0:1])
            nc.scalar.dma_start(out=att_d[i * P : (i + 1) * P, :], in_=att_t)

            # out = h + elu(h' * rec); elu(x) = min(exp(x),1) - 1 + relu(x)
            xe = elup.tile([P, F], F32, tag="xe")
            nc.scalar.activation(
                xe, hp[:, 0:F], AF.Exp, bias=0.0, scale=rec[:, 0:1]
            )
            xp = elup.tile([P, F], F32, tag="xp")
            nc.vector.tensor_scalar(
                out=xp, in0=hp[:, 0:F], scalar1=rec[:, 0:1], scalar2=0.0,
                op0=OP.mult, op1=OP.max,
            )
            tm = elup.tile([P, F], F32, tag="tm")
            nc.vector.tensor_scalar(
                out=tm, in0=xe, scalar1=1.0, scalar2=-1.0,
                op0=OP.min, op1=OP.add,
            )
            s1 = elup.tile([P, F], F32, tag="s1")
            nc.vector.tensor_tensor(out=s1, in0=tm, in1=xp, op=OP.add)
            ou = elup.tile([P, F], F32, tag="ou")
            nc.gpsimd.tensor_tensor(
                out=ou, in0=s1, in1=h_sb[:, i * F : (i + 1) * F], op=OP.add
            )
            nc.gpsimd.dma_start(out=out_d[i * P : (i + 1) * P, :], in_=ou)

    _split_sync_waits(nc)
    return nc


_NC = None


def kernel(h, adj, W, a):
    global _NC
    if _NC is None:
        _NC = _build()
    h = np.ascontiguousarray(np.asarray(h, dtype=np.float32))
    adj = np.asarray(adj)
    W = np.asarray(W, dtype=np.float32)
    a = np.asarray(a, dtype=np.float32)
    wext = np.concatenate([W, W @ a[:F], W @ a[F:]], axis=1).astype(np.float32)
    ident = np.eye(P, dtype=np.float32)
    in_maps = [
        {
            "h": h[b],
            "hT": np.ascontiguousarray(h[b].T),
            "adj8": np.ascontiguousarray((adj[b] > 0).astype(np.uint8)),
            "wext": wext,
            "ident": ident,
        }
        for b in range(B)
    ]
    res = run_bass_kernel_spmd(_NC, in_maps, list(range(B)))
    out = np.stack([res.results[b]["out"] for b in range(B)])
    att = np.stack([res.results[b]["att"] for b in range(B)])
    return out, att
